# revision 44
# baseline (speedup 1.0000x reference)
"""Sliding-window (banded) multi-head self-attention on 8 trn2 NeuronCores.

Sequence-parallel sharding: batch b, 2048 tokens -> 4 chunks of 512 queries;
core c handles batch c//4, chunk c%4.  Each core receives x^T for its 512
tokens plus a 128-token halo (zero-padded for chunk 0), computes
qkv projection + RoPE + banded attention (window 129) + out projection for
its rows, and returns [512, 2048].  No cross-core communication.

v3 dtype strategy (from measured PE stream rates: fp32r "HIGH" mode moves
0.287 ns/col vs bf16's 0.427 ns/col, but fp32r LDWEIGHTS is ~2x pricier and
fp32 DMA is 2x the bytes; mixing 16/32-bit matmul operands is rejected by
walrus):
  - QKV projections (N=512/320 chains, stream-bound): fp32r end-to-end.
  - attention score/PV/rowsum matmuls (N=256, LDW-sensitive): bf16.
  - out projection + its weights (DMA-budget relief): bf16.
Every bulk DMA is split into <=330KB pieces - each hardware DMA engine
moves only ~20GB/s, so big monolithic transfers serialize behind one
engine while 16 run in parallel.  Weights prefetch ~1.5 heads ahead.
Output is written bf16; host converts and adds the bias.
"""

import math
import numpy as np
import ml_dtypes

import concourse.bass as bass
import concourse.tile as tile
from concourse import mybir
from concourse.bass_utils import run_bass_kernel_spmd
from concourse.vector_clock import ScopedClock, VectorClock


def _legalize_single_wait(nc):
    """This walrus build accepts only ONE sync-wait per lowered command
    ("Too many sync wait commands").  Move all but the last wait of every
    instruction onto single-wait NoOps prepended on the same engine: engines
    are in-order, so stalling on the NoOps is equivalent.  SP-issued DMAs are
    gated the same way (descriptor push happens in SP program order)."""
    nid = [0]
    for f in nc.m.functions:
        for blk in f.blocks:
            out = []
            changed = False
            for inst in blk.instructions:
                si = inst.sync_info
                waits = list(si.on_wait) if si and si.on_wait else []
                if len(waits) > 1:
                    changed = True
                    for w in waits[:-1]:
                        nop = mybir.InstNoOp(name=f"waitnop-{nid[0]}", ins=[], outs=[])
                        nid[0] += 1
                        nop.engine = inst.engine
                        nop.sync_info = mybir.SyncInfo(on_wait=[w], on_update=[])
                        out.append(nop)
                    inst.sync_info = mybir.SyncInfo(
                        on_wait=[waits[-1]], on_update=list(si.on_update or [])
                    )
                out.append(inst)
            if changed:
                blk.instructions = out
    return nc


def _install_drain_split_patch():
    """Split TileContext's closing drain into single-wait drains: walrus's
    CTRL_NO command rejects the catch-all drain ("Too many sync waits")."""
    if getattr(tile.TileContext, "_drain_split_patched", False):
        return

    def _patched(self, tick_clock, wait_clock):
        gvc = tick_clock.global_clock  # VectorClock over the 27 procs
        n = len(gvc)
        procs = [i for i in range(n) if gvc[i] > 0]
        for pi in procs:
            vc = VectorClock([gvc[i] if i == pi else 0 for i in range(n)])
            d = self.nc.sync.drain()
            wait_clock.add_sem_waits(d.ins, ScopedClock({None: vc}))
        self.nc.all_engine_barrier()
        assert self.sems is not None
        popped = self.nc._tile_sem_poison_stack.pop()
        assert popped is self._sem_poison
        self.nc.clear_and_free_semaphores(list(self.sems.allocated().values()))
        self.nc.all_engine_barrier()

    tile.TileContext._drain_and_barrier = _patched
    tile.TileContext._drain_split_patched = True


_install_drain_split_patch()

EMBED = 2048
HEADS = 16
HD = 128
WINDOW = 128
THETA = 10000.0
B = 2
L = 2048
S = 512            # queries per core
T = S + WINDOW     # k/v tokens per core (incl halo)
NCORES = 8
P = 128
F32 = mybir.dt.float32
F32R = mybir.dt.float32r
BF16 = mybir.dt.bfloat16
BFNP = ml_dtypes.bfloat16


def build_bass(legalize=True):
    nc = bass.Bass("TRN2", target_bir_lowering=False, debug=False)

    XTB = nc.dram_tensor("XTB", [P, EMBED // P, T], BF16, kind="ExternalInput")
    WQ = nc.dram_tensor("WQ", [HEADS, P, EMBED // P, HD], BF16, kind="ExternalInput")
    WK = nc.dram_tensor("WK", [HEADS, P, EMBED // P, HD], BF16, kind="ExternalInput")
    WV = nc.dram_tensor("WV", [4, P, EMBED // P, 512], BF16, kind="ExternalInput")
    WO = nc.dram_tensor("WO", [4, P, EMBED // P, 512], BF16, kind="ExternalInput")
    COSQ = nc.dram_tensor("COSQ", [P, S], BF16, kind="ExternalInput")
    SINQ = nc.dram_tensor("SINQ", [P, S], BF16, kind="ExternalInput")
    COSK = nc.dram_tensor("COSK", [P, T], BF16, kind="ExternalInput")
    SINK = nc.dram_tensor("SINK", [P, T], BF16, kind="ExternalInput")
    MASKS = nc.dram_tensor("MASKS", [4, P, 256], BF16, kind="ExternalInput")
    ONES = nc.dram_tensor("ONES", [P, P], BF16, kind="ExternalInput")
    OUT = nc.dram_tensor("OUT", [S, EMBED], BF16, kind="ExternalOutput")

    EC = EMBED // P  # 16 e-chunks
    GROUPS = 4       # head groups of 4 (for V projection at N=512)
    GH = HEADS // GROUPS

    with tile.TileContext(nc) as tc:
        with (
            tc.tile_pool(name="persist", bufs=1) as persist,
            tc.tile_pool(name="wq_p", bufs=6) as wq_p,
            tc.tile_pool(name="wk_p", bufs=6) as wk_p,
            tc.tile_pool(name="wv_p", bufs=8) as wv_p,
            tc.tile_pool(name="wo_p", bufs=3) as wo_p,
            tc.tile_pool(name="rope", bufs=2) as rope,
            tc.tile_pool(name="vsb", bufs=8) as vsb_pool,
            tc.tile_pool(name="attn", bufs=4) as attn_pool,
            tc.tile_pool(name="small", bufs=2) as small,
            tc.tile_pool(name="outsb", bufs=2) as outsb,
            tc.tile_pool(name="ps_qv", bufs=2, space="PSUM") as ps_qv,
            tc.tile_pool(name="ps_k", bufs=1, space="PSUM") as ps_k,
            tc.tile_pool(name="ps_sc", bufs=2, space="PSUM") as ps_sc,
            tc.tile_pool(name="ps_oc", bufs=2, space="PSUM") as ps_oc,
        ):
            # ---- weight prefetch helpers (all bulk loads in <=330KB DMAs) --
            def load_wq(h, split=2):
                """One [P,16,HD] tile per head, in `split` pieces so they ride
                parallel ~20GB/s DMA engines (head 0 uses 4 for startup)."""
                t_ = wq_p.tile([P, 16, HD], BF16, tag="wq", name=f"wq{h}")
                step = 16 // split
                for i in range(split):
                    nc.sync.dma_start(
                        t_[:, i * step : (i + 1) * step, :],
                        WQ.ap()[h, :, i * step : (i + 1) * step, :],
                    )
                return t_

            def load_wk(h, split=2):
                t_ = wk_p.tile([P, 16, HD], BF16, tag="wk", name=f"wk{h}")
                step = 16 // split
                for i in range(split):
                    nc.sync.dma_start(
                        t_[:, i * step : (i + 1) * step, :],
                        WK.ap()[h, :, i * step : (i + 1) * step, :],
                    )
                return t_

            def load_wv_tile(g, q):
                t_ = wv_p.tile([P, 4, 512], BF16, tag="wv", name=f"wv{g}_{q}")
                nc.sync.dma_start(t_, WV.ap()[g, :, 4 * q : 4 * q + 4, :])
                return t_

            # ---- startup: head-0 weights first (split for parallel DMA
            # engines), then x chunks, then group-0 V weights + head 1 ----
            wq_next = load_wq(0, split=4)
            wk_next = load_wk(0, split=4)

            xtb = []
            for ec in range(EC):
                t_ = persist.tile([P, T], BF16, tag=f"xtb{ec}", name=f"xtb{ec}")
                nc.sync.dma_start(t_, XTB.ap()[:, ec, :])
                xtb.append(t_)

            wv_first = [load_wv_tile(0, q) for q in range(4)]
            wq_next2 = load_wq(1)
            wk_next2 = load_wk(1)

            cosq = persist.tile([P, S], BF16, tag="cosq")
            sinq = persist.tile([P, S], BF16, tag="sinq")
            cosk = persist.tile([P, T], BF16, tag="cosk")
            sink = persist.tile([P, T], BF16, tag="sink")
            masks = persist.tile([P, 4, 256], BF16, tag="masks")
            ones_full = persist.tile([P, P], BF16, tag="ones_full")

            def emit_small_loads():
                nc.sync.dma_start(cosq, COSQ.ap())
                nc.sync.dma_start(sinq, SINQ.ap())
                nc.sync.dma_start(cosk, COSK.ap())
                nc.sync.dma_start(sink, SINK.ap())
                nc.sync.dma_start(masks, MASKS.ap().rearrange("m p q -> p m q"))
                nc.sync.dma_start(ones_full, ONES.ap())

            out_norm = persist.tile([P, HEADS, S], BF16, tag="out_norm")

            # ---- per-head compute, software-pipelined: project head h+1
            # before attending head h so PE never stalls on the RoPE chain ----
            HT = T // 2  # 320

            def emit_vproj(g, wv_tiles):
                v_tiles = []
                for tt in range(T // P):  # 5 token tiles
                    psv = ps_qv.tile([P, 512], F32, tag="psqv", name=f"psv{g}_{tt}")
                    for ec in range(EC):
                        nc.tensor.matmul(
                            psv,
                            xtb[ec][:, tt * P : (tt + 1) * P],
                            wv_tiles[ec // 4][:, ec % 4, :],
                            start=(ec == 0),
                            stop=(ec == EC - 1),
                        )
                    v_sb = vsb_pool.tile([P, 512], BF16, tag="vsb", name=f"v{g}_{tt}")
                    nc.scalar.copy(v_sb, psv)
                    v_tiles.append(v_sb)
                return v_tiles

            def emit_proj(h, wq_t, wk_t, prefetch):
                """Q/K projection + RoPE for head h.  prefetch() is called
                between the Q and K chains to issue the next head's loads."""
                psq = ps_qv.tile([P, S], F32, tag="psqv", name=f"psq{h}")
                for ec in range(EC):
                    w = wq_t[:, ec, :]
                    nc.tensor.matmul(
                        psq,
                        w,
                        xtb[ec][:, WINDOW:T],
                        start=(ec == 0),
                        stop=(ec == EC - 1),
                    )
                q_sb = rope.tile([P, S], BF16, tag="qrope", name=f"q{h}")
                qraw_t = rope.tile([P, T], BF16, tag="raw", name=f"qraw{h}")
                qraw = qraw_t[:, 0:S]
                nc.scalar.copy(qraw, psq)
                nc.vector.tensor_mul(q_sb, psq, cosq)
                qsw_t = rope.tile([P, T], BF16, tag="sw", name=f"qsw{h}")
                qsw = qsw_t[:, 0:S]
                # swaps are issued from the ACT hwdge queue: their wait (the
                # qraw copy, also on ACT) is in-order-satisfied there, so the
                # SP queue never head-of-line blocks on them
                nc.scalar.dma_start(qsw[0:64, :], qraw[64:128, :])
                nc.scalar.dma_start(qsw[64:128, :], qraw[0:64, :])
                nc.gpsimd.tensor_mul(qsw, qsw, sinq)
                nc.vector.tensor_add(q_sb, q_sb, qsw)

                prefetch()

                # K projection + RoPE (tokens 0..640 in two 320 halves)
                psk1 = ps_k.tile([P, HT], F32, tag="psk1", name=f"psk1_{h}")
                psk2 = ps_k.tile([P, HT], F32, tag="psk2", name=f"psk2_{h}")
                for ec in range(EC):
                    w = wk_t[:, ec, :]
                    nc.tensor.matmul(
                        psk1,
                        w,
                        xtb[ec][:, 0:HT],
                        start=(ec == 0),
                        stop=(ec == EC - 1),
                    )
                for ec in range(EC):
                    w = wk_t[:, ec, :]
                    nc.tensor.matmul(
                        psk2,
                        w,
                        xtb[ec][:, HT:T],
                        start=(ec == 0),
                        stop=(ec == EC - 1),
                    )
                k_sb = rope.tile([P, T], BF16, tag="krope", name=f"k{h}")
                kraw = rope.tile([P, T], BF16, tag="raw", name=f"kraw{h}")
                nc.scalar.copy(kraw[:, 0:HT], psk1)
                nc.scalar.copy(kraw[:, HT:T], psk2)
                nc.vector.tensor_mul(k_sb[:, 0:HT], psk1, cosk[:, 0:HT])
                nc.vector.tensor_mul(k_sb[:, HT:T], psk2, cosk[:, HT:T])
                ksw = rope.tile([P, T], BF16, tag="sw", name=f"ksw{h}")
                nc.scalar.dma_start(ksw[0:64, :], kraw[64:128, :])
                nc.scalar.dma_start(ksw[64:128, :], kraw[0:64, :])
                nc.gpsimd.tensor_mul(ksw, ksw, sink)
                nc.vector.tensor_add(k_sb, k_sb, ksw)
                return q_sb, k_sb

            def emit_attn(h, q_sb, k_sb, v_tiles):
                hh = h % GH
                for p in range(2):
                    qs = p * 256
                    ets = []
                    for j in range(3):  # roles R1,R2,R3 -> k-chunk 2p+j
                        c = 2 * p + j
                        midx = 3 if (j == 0 and p == 1) else j
                        psc = ps_sc.tile([P, 256], F32, tag="sc", name=f"sc{h}_{p}{j}")
                        nc.tensor.matmul(
                            psc,
                            k_sb[:, c * P : (c + 1) * P],
                            q_sb[:, qs : qs + 256],
                            start=True,
                            stop=True,
                        )
                        et = attn_pool.tile(
                            [P, 256], BF16, tag="attn", name=f"et{h}_{p}{j}"
                        )
                        nc.scalar.activation(
                            et, psc, mybir.ActivationFunctionType.Exp
                        )
                        nc.gpsimd.tensor_mul(et, et, masks[:, midx, :])
                        ets.append(et)

                    poc = ps_oc.tile([P, 512], F32, tag="oc", name=f"poc{h}_{p}")
                    for j in range(3):
                        c = 2 * p + j
                        nc.tensor.matmul(
                            poc[:, 0:256],
                            v_tiles[c][:, hh * HD : (hh + 1) * HD],
                            ets[j],
                            start=(j == 0),
                            stop=False,
                        )
                        nc.tensor.matmul(
                            poc[:, 256:512],
                            ones_full,
                            ets[j],
                            start=False,
                            stop=(j == 2),
                        )
                    recip = small.tile([P, 256], F32, tag="recip", name=f"rc{h}_{p}")
                    nc.vector.reciprocal(recip, poc[:, 256:512])
                    nc.vector.tensor_mul(
                        out_norm[:, h, qs : qs + 256], poc[:, 0:256], recip
                    )

            emit_small_loads()
            v_groups = {}
            wv_next = wv_first
            pending = None
            wv_tiles0 = None
            for g in range(GROUPS):
                h0 = g * GH
                # all four tiles were prefetched a group ahead
                wv_tiles = wv_next
                if g == 0:
                    wv_tiles0 = wv_tiles

                for hh in range(GH):
                    h = g * GH + hh
                    wq_cur, wk_cur = wq_next, wk_next
                    wq_next, wk_next = wq_next2, wk_next2

                    def prefetch(h=h, g=g, hh=hh):
                        nonlocal wq_next2, wk_next2, wv_next
                        if h + 2 < HEADS:
                            wq_next2 = load_wq(h + 2)
                            wk_next2 = load_wk(h + 2)
                        if hh == GH - 2 and g + 1 < GROUPS:
                            wv_next = [load_wv_tile(g + 1, q) for q in range(4)]

                    qk = emit_proj(h, wq_cur, wk_cur, prefetch)
                    if g == 0 and hh == 1:
                        # group 0's V-proj is deferred one head so its x/wv
                        # fp32 streams have time to land
                        v_groups[0] = emit_vproj(0, wv_tiles0)
                    if pending is not None:
                        ph = pending[0]
                        emit_attn(ph, pending[1], pending[2], v_groups[ph // GH])
                    if hh == 0 and g > 0:
                        v_groups[g] = emit_vproj(g, wv_tiles)
                    pending = (h, qk[0], qk[1])

            # prefetch first out-projection weights, then last attention
            def load_wo(eo):
                a = wo_p.tile([P, 8, 512], BF16, tag="wo", name=f"wo{eo}a")
                b = wo_p.tile([P, 8, 512], BF16, tag="wo", name=f"wo{eo}b")
                for i in range(4):
                    nc.sync.dma_start(
                        a[:, 2 * i : 2 * i + 2, :],
                        WO.ap()[eo, :, 2 * i : 2 * i + 2, :],
                    )
                    nc.sync.dma_start(
                        b[:, 2 * i : 2 * i + 2, :],
                        WO.ap()[eo, :, 8 + 2 * i : 10 + 2 * i, :],
                    )
                return a, b

            wo_cur = load_wo(0)
            ph = pending[0]
            emit_attn(ph, pending[1], pending[2], v_groups[ph // GH])

            # ---- out projection: OUT[t, e] = sum_hd out_norm^T . WO ----
            for eo in range(4):
                e0 = eo * 512
                woa, wob = wo_cur
                if eo < 3:
                    wo_next = load_wo(eo + 1)
                for tt in range(4):
                    pso = ps_oc.tile([P, 512], F32, tag="oc")
                    for hd in range(HEADS):
                        w = woa[:, hd, :] if hd < 8 else wob[:, hd - 8, :]
                        nc.tensor.matmul(
                            pso,
                            out_norm[:, hd, tt * P : (tt + 1) * P],
                            w,
                            start=(hd == 0),
                            stop=(hd == HEADS - 1),
                        )
                    o_sb = outsb.tile([P, 512], BF16, tag="osb")
                    nc.scalar.copy(o_sb, pso)
                    nc.scalar.dma_start(
                        OUT.ap()[tt * P : (tt + 1) * P, e0 : e0 + 256],
                        o_sb[:, 0:256],
                    )
                    nc.scalar.dma_start(
                        OUT.ap()[tt * P : (tt + 1) * P, e0 + 256 : e0 + 512],
                        o_sb[:, 256:512],
                    )
                if eo < 3:
                    wo_cur = wo_next

    if legalize:
        _legalize_single_wait(nc)
    return nc


def _rope_tables(pos, scale):
    """Feature-major [128, len(pos)] cos / sin' tables in de-interleaved d order.

    cos'[i, t] = cos(pos_t * invf[i % 64]) ; sin'[0:64] = -sin, sin'[64:128] = +sin.
    """
    inv_freq = 1.0 / (THETA ** (np.arange(0, HD, 2, dtype=np.float64) / HD))  # [64]
    ang = pos[None, :] * inv_freq[:, None]  # [64, T]
    cos = np.cos(ang)
    sin = np.sin(ang)
    cos_t = np.concatenate([cos, cos], axis=0) * scale
    sin_t = np.concatenate([-sin, sin], axis=0) * scale
    return cos_t.astype(BFNP), sin_t.astype(BFNP)


def _band_masks(start):
    """[4, 128, 256] multiplicative masks.

    Element (m, kp, qf): role m in {R1 pair0, R2, R3, R1 pair1};
    local key j = c*128 + kp, local query r = qs + qf;
    valid iff r <= j <= r + 128 and (global key) start - 128 + j >= 0.
    """
    out = np.zeros((4, P, 256), dtype=BFNP)
    roles = [(0, 0), (1, 0), (2, 0), (2, 256)]  # (chunk c, query offset qs)
    for m, (c, qs) in enumerate(roles):
        kp = np.arange(P)[:, None]
        qf = np.arange(256)[None, :]
        j = c * P + kp
        r = qs + qf
        valid = (r <= j) & (j <= r + WINDOW) & (start - WINDOW + j >= 0)
        out[m] = valid.astype(BFNP)
    return out


_CACHED = {}
LAST_RESULT = {}


def prepare_in_maps(x, W_qkv, W_out, b_out):
    x = np.asarray(x, dtype=np.float32)
    W_qkv = np.asarray(W_qkv, dtype=np.float32)
    W_out = np.asarray(W_out, dtype=np.float32)

    # host-side weight layout prep
    perm = np.concatenate([np.arange(0, HD, 2), np.arange(1, HD, 2)])  # de-interleave
    w4 = W_qkv.reshape(EMBED, HEADS, HD, 3)
    # [h, e, d] -> [h, p, ec, d] partition-major contiguous
    WQ = w4[..., 0].transpose(1, 0, 2)[:, :, perm].reshape(HEADS, EMBED // P, P, HD)
    WQ = np.ascontiguousarray(WQ.transpose(0, 2, 1, 3)).astype(BFNP)
    WK = w4[..., 1].transpose(1, 0, 2)[:, :, perm].reshape(HEADS, EMBED // P, P, HD)
    WK = np.ascontiguousarray(WK.transpose(0, 2, 1, 3)).astype(BFNP)
    # [e, f] -> [g, p, ec, 512]
    WV = w4[..., 2].reshape(EMBED // P, P, 4, 512)
    WV = np.ascontiguousarray(WV.transpose(2, 1, 0, 3)).astype(BFNP)
    WOa = W_out.reshape(EMBED // P, P, 4, 512)
    WOa = np.ascontiguousarray(WOa.transpose(2, 1, 0, 3)).astype(BFNP)

    in_maps = []
    for core in range(NCORES):
        b = core // 4
        start = (core % 4) * S
        # x^T with halo, zero-padded at the left for chunk 0
        xt = np.zeros((EMBED, T), dtype=np.float32)
        lo = start - WINDOW
        src = x[b, max(lo, 0) : start + S, :]  # [<=640, e]
        xt[:, T - src.shape[0] :] = src.T
        xt = np.ascontiguousarray(xt.reshape(EMBED // P, P, T).transpose(1, 0, 2))
        # rope tables: query positions start..start+512, key positions lo..start+512
        qpos = np.arange(start, start + S, dtype=np.float64)
        kpos = np.maximum(np.arange(lo, start + S, dtype=np.float64), 0.0)
        scale = 1.0 / math.sqrt(HD)
        cq, sq = _rope_tables(qpos, scale)
        ck, sk = _rope_tables(kpos, 1.0)
        in_maps.append(
            {
                "XTB": xt.astype(BFNP),
                "WQ": WQ,
                "WK": WK,
                "WV": WV,
                "WO": WOa,
                "COSQ": cq,
                "SINQ": sq,
                "COSK": ck,
                "SINK": sk,
                "MASKS": _band_masks(start),
                "ONES": np.ones((P, P), dtype=BFNP),
            }
        )
    return in_maps


def kernel(x, W_qkv, W_out, b_out):
    b_out = np.asarray(b_out, dtype=np.float32)
    in_maps = prepare_in_maps(x, W_qkv, W_out, b_out)

    if "nc" not in _CACHED:
        _CACHED["nc"] = build_bass()
    nc = _CACHED["nc"]

    res = run_bass_kernel_spmd(nc, in_maps, core_ids=list(range(NCORES)))
    LAST_RESULT["res"] = res

    out = np.empty((B, L, EMBED), dtype=np.float32)
    for core in range(NCORES):
        b = core // 4
        start = (core % 4) * S
        out[b, start : start + S, :] = (
            np.asarray(res.results[core]["OUT"]).astype(np.float32) + b_out[None, :]
        )
    return out


# revision 51
# speedup vs baseline: 1.0958x; 1.0958x over previous
"""Sliding-window (banded) multi-head self-attention on 8 trn2 NeuronCores.

Sequence-parallel sharding: batch b, 2048 tokens -> 4 chunks of 512 queries;
core c handles batch c//4, chunk c%4.  Each core receives x^T for its 512
tokens plus a 128-token halo (zero-padded for chunk 0), computes
qkv projection + RoPE + banded attention (window 129) + out projection for
its rows, and returns [512, 2048].  No cross-core communication.

v3 dtype strategy (from measured PE stream rates: fp32r "HIGH" mode moves
0.287 ns/col vs bf16's 0.427 ns/col, but fp32r LDWEIGHTS is ~2x pricier and
fp32 DMA is 2x the bytes; mixing 16/32-bit matmul operands is rejected by
walrus):
  - QKV projections (N=512/320 chains, stream-bound): fp32r end-to-end.
  - attention score/PV/rowsum matmuls (N=256, LDW-sensitive): bf16.
  - out projection + its weights (DMA-budget relief): bf16.
Every bulk DMA is split into <=330KB pieces - each hardware DMA engine
moves only ~20GB/s, so big monolithic transfers serialize behind one
engine while 16 run in parallel.  Weights prefetch ~1.5 heads ahead.
Output is written bf16; host converts and adds the bias.
"""

import math
import numpy as np
import ml_dtypes

import concourse.bass as bass
import concourse.tile as tile
from concourse import mybir
from concourse.bass_utils import run_bass_kernel_spmd
from concourse.vector_clock import ScopedClock, VectorClock


def _legalize_single_wait(nc):
    """This walrus build accepts only ONE sync-wait per lowered command
    ("Too many sync wait commands").  Move all but the last wait of every
    instruction onto single-wait NoOps prepended on the same engine: engines
    are in-order, so stalling on the NoOps is equivalent.  SP-issued DMAs are
    gated the same way (descriptor push happens in SP program order)."""
    nid = [0]
    for f in nc.m.functions:
        for blk in f.blocks:
            out = []
            changed = False
            for inst in blk.instructions:
                si = inst.sync_info
                waits = list(si.on_wait) if si and si.on_wait else []
                if len(waits) > 1:
                    changed = True
                    for w in waits[:-1]:
                        nop = mybir.InstNoOp(name=f"waitnop-{nid[0]}", ins=[], outs=[])
                        nid[0] += 1
                        nop.engine = inst.engine
                        nop.sync_info = mybir.SyncInfo(on_wait=[w], on_update=[])
                        out.append(nop)
                    inst.sync_info = mybir.SyncInfo(
                        on_wait=[waits[-1]], on_update=list(si.on_update or [])
                    )
                out.append(inst)
            if changed:
                blk.instructions = out
    return nc


def _install_drain_split_patch():
    """Split TileContext's closing drain into single-wait drains: walrus's
    CTRL_NO command rejects the catch-all drain ("Too many sync waits")."""
    if getattr(tile.TileContext, "_drain_split_patched", False):
        return

    def _patched(self, tick_clock, wait_clock):
        gvc = tick_clock.global_clock  # VectorClock over the 27 procs
        n = len(gvc)
        procs = [i for i in range(n) if gvc[i] > 0]
        for pi in procs:
            vc = VectorClock([gvc[i] if i == pi else 0 for i in range(n)])
            d = self.nc.sync.drain()
            wait_clock.add_sem_waits(d.ins, ScopedClock({None: vc}))
        self.nc.all_engine_barrier()
        assert self.sems is not None
        popped = self.nc._tile_sem_poison_stack.pop()
        assert popped is self._sem_poison
        self.nc.clear_and_free_semaphores(list(self.sems.allocated().values()))
        self.nc.all_engine_barrier()

    tile.TileContext._drain_and_barrier = _patched
    tile.TileContext._drain_split_patched = True


_install_drain_split_patch()

EMBED = 2048
HEADS = 16
HD = 128
WINDOW = 128
THETA = 10000.0
B = 2
L = 2048
S = 512            # queries per core
T = S + WINDOW     # k/v tokens per core (incl halo)
NCORES = 8
P = 128
F32 = mybir.dt.float32
F32R = mybir.dt.float32r
BF16 = mybir.dt.bfloat16
BFNP = ml_dtypes.bfloat16


def build_bass(legalize=True):
    nc = bass.Bass("TRN2", target_bir_lowering=False, debug=False)

    XTB = nc.dram_tensor("XTB", [P, EMBED // P, T], BF16, kind="ExternalInput")
    WQ = nc.dram_tensor("WQ", [HEADS, P, EMBED // P, HD], BF16, kind="ExternalInput")
    WK = nc.dram_tensor("WK", [HEADS, P, EMBED // P, HD], BF16, kind="ExternalInput")
    WV = nc.dram_tensor("WV", [4, P, EMBED // P, 512], BF16, kind="ExternalInput")
    WO = nc.dram_tensor("WO", [4, P, EMBED // P, 512], BF16, kind="ExternalInput")
    COSQ = nc.dram_tensor("COSQ", [P, S], BF16, kind="ExternalInput")
    SINQ = nc.dram_tensor("SINQ", [P, S], BF16, kind="ExternalInput")
    COSK = nc.dram_tensor("COSK", [P, T], BF16, kind="ExternalInput")
    SINK = nc.dram_tensor("SINK", [P, T], BF16, kind="ExternalInput")
    MASKS = nc.dram_tensor("MASKS", [4, P, 256], BF16, kind="ExternalInput")
    ONES = nc.dram_tensor("ONES", [P, P], BF16, kind="ExternalInput")
    OUT = nc.dram_tensor("OUT", [S, EMBED], BF16, kind="ExternalOutput")

    EC = EMBED // P  # 16 e-chunks
    GROUPS = 4       # head groups of 4 (for V projection at N=512)
    GH = HEADS // GROUPS

    with tile.TileContext(nc) as tc:
        with (
            tc.tile_pool(name="persist", bufs=1) as persist,
            tc.tile_pool(name="wq_p", bufs=6) as wq_p,
            tc.tile_pool(name="wk_p", bufs=6) as wk_p,
            tc.tile_pool(name="wv_p", bufs=8) as wv_p,
            tc.tile_pool(name="wo_p", bufs=3) as wo_p,
            tc.tile_pool(name="rope", bufs=2) as rope,
            tc.tile_pool(name="vsb", bufs=8) as vsb_pool,
            tc.tile_pool(name="attn", bufs=4) as attn_pool,
            tc.tile_pool(name="small", bufs=2) as small,
            tc.tile_pool(name="outsb", bufs=2) as outsb,
            tc.tile_pool(name="ps_qv", bufs=2, space="PSUM") as ps_qv,
            tc.tile_pool(name="ps_k", bufs=1, space="PSUM") as ps_k,
            tc.tile_pool(name="ps_sc", bufs=2, space="PSUM") as ps_sc,
            tc.tile_pool(name="ps_oc", bufs=2, space="PSUM") as ps_oc,
        ):
            # ---- weight prefetch helpers (all bulk loads in <=330KB DMAs) --
            def load_wq(h, split=1):
                """Two [P,8,HD] tiles per head (piece-wise chain deps: the
                first 8 accumulation steps only wait on the first ~260KB)."""
                a = wq_p.tile([P, 8, HD], BF16, tag="wq", name=f"wq{h}a")
                b = wq_p.tile([P, 8, HD], BF16, tag="wq", name=f"wq{h}b")
                for t_, off in ((a, 0), (b, 8)):
                    step = 8 // split
                    for i in range(split):
                        nc.sync.dma_start(
                            t_[:, i * step : (i + 1) * step, :],
                            WQ.ap()[h, :, off + i * step : off + (i + 1) * step, :],
                        )
                return a, b

            def load_wk(h, split=1):
                a = wk_p.tile([P, 8, HD], BF16, tag="wk", name=f"wk{h}a")
                b = wk_p.tile([P, 8, HD], BF16, tag="wk", name=f"wk{h}b")
                for t_, off in ((a, 0), (b, 8)):
                    step = 8 // split
                    for i in range(split):
                        nc.sync.dma_start(
                            t_[:, i * step : (i + 1) * step, :],
                            WK.ap()[h, :, off + i * step : off + (i + 1) * step, :],
                        )
                return a, b

            def load_wv_tile(g, q):
                t_ = wv_p.tile([P, 4, 512], BF16, tag="wv", name=f"wv{g}_{q}")
                nc.sync.dma_start(t_, WV.ap()[g, :, 4 * q : 4 * q + 4, :])
                return t_

            # ---- startup: head-0 weights first (split for parallel DMA
            # engines), then x chunks, then group-0 V weights + head 1 ----
            wq_next = load_wq(0, split=2)
            wk_next = load_wk(0, split=2)

            xtb = []
            for ec in range(EC):
                t_ = persist.tile([P, T], BF16, tag=f"xtb{ec}", name=f"xtb{ec}")
                nc.sync.dma_start(t_, XTB.ap()[:, ec, :])
                xtb.append(t_)

            wv_first = [load_wv_tile(0, q) for q in range(4)]
            wq_next2 = load_wq(1)
            wk_next2 = load_wk(1)

            cosq = persist.tile([P, S], BF16, tag="cosq")
            sinq = persist.tile([P, S], BF16, tag="sinq")
            cosk = persist.tile([P, T], BF16, tag="cosk")
            sink = persist.tile([P, T], BF16, tag="sink")
            masks = persist.tile([P, 4, 256], BF16, tag="masks")
            ones_full = persist.tile([P, P], BF16, tag="ones_full")

            def emit_small_loads():
                nc.sync.dma_start(cosq, COSQ.ap())
                nc.sync.dma_start(sinq, SINQ.ap())
                nc.sync.dma_start(cosk, COSK.ap())
                nc.sync.dma_start(sink, SINK.ap())
                nc.sync.dma_start(masks, MASKS.ap().rearrange("m p q -> p m q"))
                nc.sync.dma_start(ones_full, ONES.ap())

            out_norm = persist.tile([P, HEADS, S], BF16, tag="out_norm")

            # ---- per-head compute, software-pipelined: project head h+1
            # before attending head h so PE never stalls on the RoPE chain ----
            HT = T // 2  # 320

            def emit_vproj(g, wv_tiles):
                v_tiles = []
                for tt in range(T // P):  # 5 token tiles
                    psv = ps_qv.tile([P, 512], F32, tag="psqv", name=f"psv{g}_{tt}")
                    for ec in range(EC):
                        nc.tensor.matmul(
                            psv,
                            xtb[ec][:, tt * P : (tt + 1) * P],
                            wv_tiles[ec // 4][:, ec % 4, :],
                            start=(ec == 0),
                            stop=(ec == EC - 1),
                        )
                    v_sb = vsb_pool.tile([P, 512], BF16, tag="vsb", name=f"v{g}_{tt}")
                    nc.scalar.copy(v_sb, psv)
                    v_tiles.append(v_sb)
                return v_tiles

            def emit_proj(h, wq_t, wk_t, prefetch):
                """Q/K projection + RoPE for head h.  prefetch() is called
                between the Q and K chains to issue the next head's loads."""
                wqa, wqb = wq_t
                psq = ps_qv.tile([P, S], F32, tag="psqv", name=f"psq{h}")
                for ec in range(EC):
                    w = wqa[:, ec, :] if ec < 8 else wqb[:, ec - 8, :]
                    nc.tensor.matmul(
                        psq,
                        w,
                        xtb[ec][:, WINDOW:T],
                        start=(ec == 0),
                        stop=(ec == EC - 1),
                    )
                q_sb = rope.tile([P, S], BF16, tag="qrope", name=f"q{h}")
                qraw_t = rope.tile([P, T], BF16, tag="raw", name=f"qraw{h}")
                qraw = qraw_t[:, 0:S]
                nc.scalar.copy(qraw, psq)
                nc.vector.tensor_mul(q_sb, psq, cosq)
                qsw_t = rope.tile([P, T], BF16, tag="sw", name=f"qsw{h}")
                qsw = qsw_t[:, 0:S]
                nc.sync.dma_start(qsw[0:64, :], qraw[64:128, :])
                nc.sync.dma_start(qsw[64:128, :], qraw[0:64, :])
                nc.gpsimd.tensor_mul(qsw, qsw, sinq)
                nc.vector.tensor_add(q_sb, q_sb, qsw)

                prefetch()

                # K projection + RoPE (tokens 0..640 in two 320 halves)
                wka, wkb = wk_t
                psk1 = ps_k.tile([P, HT], F32, tag="psk1", name=f"psk1_{h}")
                psk2 = ps_k.tile([P, HT], F32, tag="psk2", name=f"psk2_{h}")
                for ec in range(EC):
                    w = wka[:, ec, :] if ec < 8 else wkb[:, ec - 8, :]
                    nc.tensor.matmul(
                        psk1,
                        w,
                        xtb[ec][:, 0:HT],
                        start=(ec == 0),
                        stop=(ec == EC - 1),
                    )
                for ec in range(EC):
                    w = wka[:, ec, :] if ec < 8 else wkb[:, ec - 8, :]
                    nc.tensor.matmul(
                        psk2,
                        w,
                        xtb[ec][:, HT:T],
                        start=(ec == 0),
                        stop=(ec == EC - 1),
                    )
                k_sb = rope.tile([P, T], BF16, tag="krope", name=f"k{h}")
                kraw = rope.tile([P, T], BF16, tag="raw", name=f"kraw{h}")
                nc.scalar.copy(kraw[:, 0:HT], psk1)
                nc.scalar.copy(kraw[:, HT:T], psk2)
                nc.vector.tensor_mul(k_sb[:, 0:HT], psk1, cosk[:, 0:HT])
                nc.vector.tensor_mul(k_sb[:, HT:T], psk2, cosk[:, HT:T])
                ksw = rope.tile([P, T], BF16, tag="sw", name=f"ksw{h}")
                nc.sync.dma_start(ksw[0:64, :], kraw[64:128, :])
                nc.sync.dma_start(ksw[64:128, :], kraw[0:64, :])
                nc.gpsimd.tensor_mul(ksw, ksw, sink)
                nc.vector.tensor_add(k_sb, k_sb, ksw)
                return q_sb, k_sb

            def emit_attn(h, q_sb, k_sb, v_tiles):
                hh = h % GH
                for p in range(2):
                    qs = p * 256
                    ets = []
                    for j in range(3):  # roles R1,R2,R3 -> k-chunk 2p+j
                        c = 2 * p + j
                        midx = 3 if (j == 0 and p == 1) else j
                        psc = ps_sc.tile([P, 256], F32, tag="sc", name=f"sc{h}_{p}{j}")
                        nc.tensor.matmul(
                            psc,
                            k_sb[:, c * P : (c + 1) * P],
                            q_sb[:, qs : qs + 256],
                            start=True,
                            stop=True,
                        )
                        et = attn_pool.tile(
                            [P, 256], BF16, tag="attn", name=f"et{h}_{p}{j}"
                        )
                        nc.scalar.activation(
                            et, psc, mybir.ActivationFunctionType.Exp
                        )
                        nc.gpsimd.tensor_mul(et, et, masks[:, midx, :])
                        ets.append(et)

                    poc = ps_oc.tile([P, 512], F32, tag="oc", name=f"poc{h}_{p}")
                    for j in range(3):
                        c = 2 * p + j
                        nc.tensor.matmul(
                            poc[:, 0:256],
                            v_tiles[c][:, hh * HD : (hh + 1) * HD],
                            ets[j],
                            start=(j == 0),
                            stop=False,
                        )
                        nc.tensor.matmul(
                            poc[:, 256:512],
                            ones_full,
                            ets[j],
                            start=False,
                            stop=(j == 2),
                        )
                    recip = small.tile([P, 256], F32, tag="recip", name=f"rc{h}_{p}")
                    nc.vector.reciprocal(recip, poc[:, 256:512])
                    nc.vector.tensor_mul(
                        out_norm[:, h, qs : qs + 256], poc[:, 0:256], recip
                    )

            emit_small_loads()
            v_groups = {}
            wv_next = wv_first
            pending = None
            wv_tiles0 = None
            for g in range(GROUPS):
                h0 = g * GH
                # all four tiles were prefetched a group ahead
                wv_tiles = wv_next
                if g == 0:
                    wv_tiles0 = wv_tiles

                for hh in range(GH):
                    h = g * GH + hh
                    wq_cur, wk_cur = wq_next, wk_next
                    wq_next, wk_next = wq_next2, wk_next2

                    def prefetch(h=h, g=g, hh=hh):
                        nonlocal wq_next2, wk_next2, wv_next
                        if h + 2 < HEADS:
                            wq_next2 = load_wq(h + 2)
                            wk_next2 = load_wk(h + 2)
                        if hh == GH - 2 and g + 1 < GROUPS:
                            wv_next = [load_wv_tile(g + 1, q) for q in range(4)]

                    qk = emit_proj(h, wq_cur, wk_cur, prefetch)
                    if g == 0 and hh == 1:
                        # group 0's V-proj is deferred one head so its x/wv
                        # fp32 streams have time to land
                        v_groups[0] = emit_vproj(0, wv_tiles0)
                    if pending is not None:
                        ph = pending[0]
                        emit_attn(ph, pending[1], pending[2], v_groups[ph // GH])
                    if hh == 0 and g > 0:
                        v_groups[g] = emit_vproj(g, wv_tiles)
                    pending = (h, qk[0], qk[1])

            # prefetch first out-projection weights, then last attention
            def load_wo(eo):
                a = wo_p.tile([P, 8, 512], BF16, tag="wo", name=f"wo{eo}a")
                b = wo_p.tile([P, 8, 512], BF16, tag="wo", name=f"wo{eo}b")
                for i in range(4):
                    nc.sync.dma_start(
                        a[:, 2 * i : 2 * i + 2, :],
                        WO.ap()[eo, :, 2 * i : 2 * i + 2, :],
                    )
                    nc.sync.dma_start(
                        b[:, 2 * i : 2 * i + 2, :],
                        WO.ap()[eo, :, 8 + 2 * i : 10 + 2 * i, :],
                    )
                return a, b

            wo_cur = load_wo(0)
            ph = pending[0]
            emit_attn(ph, pending[1], pending[2], v_groups[ph // GH])

            # ---- out projection: OUT[t, e] = sum_hd out_norm^T . WO ----
            for eo in range(4):
                e0 = eo * 512
                woa, wob = wo_cur
                if eo < 3:
                    wo_next = load_wo(eo + 1)
                for tt in range(4):
                    pso = ps_oc.tile([P, 512], F32, tag="oc")
                    for hd in range(HEADS):
                        w = woa[:, hd, :] if hd < 8 else wob[:, hd - 8, :]
                        nc.tensor.matmul(
                            pso,
                            out_norm[:, hd, tt * P : (tt + 1) * P],
                            w,
                            start=(hd == 0),
                            stop=(hd == HEADS - 1),
                        )
                    o_sb = outsb.tile([P, 512], BF16, tag="osb")
                    nc.scalar.copy(o_sb, pso)
                    nc.scalar.dma_start(
                        OUT.ap()[tt * P : (tt + 1) * P, e0 : e0 + 256],
                        o_sb[:, 0:256],
                    )
                    nc.scalar.dma_start(
                        OUT.ap()[tt * P : (tt + 1) * P, e0 + 256 : e0 + 512],
                        o_sb[:, 256:512],
                    )
                if eo < 3:
                    wo_cur = wo_next

    if legalize:
        _legalize_single_wait(nc)
    return nc


def _rope_tables(pos, scale):
    """Feature-major [128, len(pos)] cos / sin' tables in de-interleaved d order.

    cos'[i, t] = cos(pos_t * invf[i % 64]) ; sin'[0:64] = -sin, sin'[64:128] = +sin.
    """
    inv_freq = 1.0 / (THETA ** (np.arange(0, HD, 2, dtype=np.float64) / HD))  # [64]
    ang = pos[None, :] * inv_freq[:, None]  # [64, T]
    cos = np.cos(ang)
    sin = np.sin(ang)
    cos_t = np.concatenate([cos, cos], axis=0) * scale
    sin_t = np.concatenate([-sin, sin], axis=0) * scale
    return cos_t.astype(BFNP), sin_t.astype(BFNP)


def _band_masks(start):
    """[4, 128, 256] multiplicative masks.

    Element (m, kp, qf): role m in {R1 pair0, R2, R3, R1 pair1};
    local key j = c*128 + kp, local query r = qs + qf;
    valid iff r <= j <= r + 128 and (global key) start - 128 + j >= 0.
    """
    out = np.zeros((4, P, 256), dtype=BFNP)
    roles = [(0, 0), (1, 0), (2, 0), (2, 256)]  # (chunk c, query offset qs)
    for m, (c, qs) in enumerate(roles):
        kp = np.arange(P)[:, None]
        qf = np.arange(256)[None, :]
        j = c * P + kp
        r = qs + qf
        valid = (r <= j) & (j <= r + WINDOW) & (start - WINDOW + j >= 0)
        out[m] = valid.astype(BFNP)
    return out


_CACHED = {}
LAST_RESULT = {}


def prepare_in_maps(x, W_qkv, W_out, b_out):
    x = np.asarray(x, dtype=np.float32)
    W_qkv = np.asarray(W_qkv, dtype=np.float32)
    W_out = np.asarray(W_out, dtype=np.float32)

    # host-side weight layout prep
    perm = np.concatenate([np.arange(0, HD, 2), np.arange(1, HD, 2)])  # de-interleave
    w4 = W_qkv.reshape(EMBED, HEADS, HD, 3)
    # [h, e, d] -> [h, p, ec, d] partition-major contiguous
    WQ = w4[..., 0].transpose(1, 0, 2)[:, :, perm].reshape(HEADS, EMBED // P, P, HD)
    WQ = np.ascontiguousarray(WQ.transpose(0, 2, 1, 3)).astype(BFNP)
    WK = w4[..., 1].transpose(1, 0, 2)[:, :, perm].reshape(HEADS, EMBED // P, P, HD)
    WK = np.ascontiguousarray(WK.transpose(0, 2, 1, 3)).astype(BFNP)
    # [e, f] -> [g, p, ec, 512]
    WV = w4[..., 2].reshape(EMBED // P, P, 4, 512)
    WV = np.ascontiguousarray(WV.transpose(2, 1, 0, 3)).astype(BFNP)
    WOa = W_out.reshape(EMBED // P, P, 4, 512)
    WOa = np.ascontiguousarray(WOa.transpose(2, 1, 0, 3)).astype(BFNP)

    in_maps = []
    for core in range(NCORES):
        b = core // 4
        start = (core % 4) * S
        # x^T with halo, zero-padded at the left for chunk 0
        xt = np.zeros((EMBED, T), dtype=np.float32)
        lo = start - WINDOW
        src = x[b, max(lo, 0) : start + S, :]  # [<=640, e]
        xt[:, T - src.shape[0] :] = src.T
        xt = np.ascontiguousarray(xt.reshape(EMBED // P, P, T).transpose(1, 0, 2))
        # rope tables: query positions start..start+512, key positions lo..start+512
        qpos = np.arange(start, start + S, dtype=np.float64)
        kpos = np.maximum(np.arange(lo, start + S, dtype=np.float64), 0.0)
        scale = 1.0 / math.sqrt(HD)
        cq, sq = _rope_tables(qpos, scale)
        ck, sk = _rope_tables(kpos, 1.0)
        in_maps.append(
            {
                "XTB": xt.astype(BFNP),
                "WQ": WQ,
                "WK": WK,
                "WV": WV,
                "WO": WOa,
                "COSQ": cq,
                "SINQ": sq,
                "COSK": ck,
                "SINK": sk,
                "MASKS": _band_masks(start),
                "ONES": np.ones((P, P), dtype=BFNP),
            }
        )
    return in_maps


def kernel(x, W_qkv, W_out, b_out):
    b_out = np.asarray(b_out, dtype=np.float32)
    in_maps = prepare_in_maps(x, W_qkv, W_out, b_out)

    if "nc" not in _CACHED:
        _CACHED["nc"] = build_bass()
    nc = _CACHED["nc"]

    res = run_bass_kernel_spmd(nc, in_maps, core_ids=list(range(NCORES)))
    LAST_RESULT["res"] = res

    out = np.empty((B, L, EMBED), dtype=np.float32)
    for core in range(NCORES):
        b = core // 4
        start = (core % 4) * S
        out[b, start : start + S, :] = (
            np.asarray(res.results[core]["OUT"]).astype(np.float32) + b_out[None, :]
        )
    return out
